# revision 59
# baseline (speedup 1.0000x reference)
"""Trainium2 Bass kernel for the attention+LN+MLP block (nn_Attention_84310208020626).

Reference computation (per batch b):
    q = x_b @ Wq.T ; k = x_b @ Wk.T ; v = x_b @ Wv.T          (S=2048, D=512)
    attn = softmax(q k^T / sqrt(512))
    res  = attn @ v
    h    = LayerNorm(res) * ln_g + ln_b
    out  = relu(h @ W1.T + b1) @ W2.T + b2

Sharding: 8 cores = 4 batches x 2 sequence halves. Every core computes its
batch's full K/V (recompute, no collectives) and runs attention + LN + MLP
for its own 1024 query rows.

Device layout: activations are feature-major [feature, seq] so that every
GEMM contracts over the partition dimension without transposes:
    GT[d',s]     = A-stationary GEMM over xT, A = Wq^T Wk precomputed on host
                   (scores = q k^T = (x A) x^T, so no separate Q/K GEMMs)
    V[t,e]       = xT-stationary GEMM (t-major, feeds the res GEMM as lhsT)
    scoresT[t,s] = xT-stationary GEMM, rhs = GT     -> exp -> expT (bf16)
    resU[e,s]    = V-stationary GEMM, rhs = expT  (softmax denom NOT applied)
    sums[1,s]    = ones-stationary GEMM over expT
LayerNorm over e (partition dim) uses ones-matmul column sums; the softmax
division is folded into LN via scale invariance with a corrected epsilon:
    LN(res) = (resU - muU) / sqrt(varU + eps*sums^2)  (exact in exact arithmetic)
and the whole LN is folded into the MLP1 GEMM epilogue:
    h1 = relu( (G1 @ res)*rstd[s] - murstd[s]*r1[f] + (W1@ln_b)[f] + b1[f] )
with G1 = W1*diag(ln_g), r1 = G1 row sums (both computed on device once).
Per-column stats are broadcast across partitions with a K=1 ones matmul.
All GEMM operands are bf16 (fp32 PSUM accumulation); LN stats math is fp32.
"""

import ml_dtypes
import numpy as np

import concourse.bass as bass
import concourse.mybir as mybir
import concourse.tile as tile
from concourse import bacc
from concourse.bass_utils import run_bass_kernel_spmd

S, B, D = 2048, 4, 512
N_CORES = 8
SQ = 1024          # query rows per core
SBLK = 512         # s-block (pipeline granularity)
NBLK = SQ // SBLK  # 2
ND = D // 128      # 4 chunks of the feature dims
NT = S // 128      # 16 t-chunks
NTT = S // 512     # 4 t-tiles of 512 for KT GEMM
EPS = 1e-5
SCALE = 1.0 / float(np.sqrt(512.0))

F32 = mybir.dt.float32
F32R = mybir.dt.float32r
BF16 = mybir.dt.bfloat16
AF = mybir.ActivationFunctionType
ALU = mybir.AluOpType


def _emit(nc, tc, n_iters=1):
    xT = nc.tensor_by_name["xT"].ap()       # (512, 2048) bf16, q-half first
    xTM = nc.tensor_by_name["xTM"].ap()     # (2048, 512) bf16, same t order
    A_qk = nc.tensor_by_name["A_qk"].ap()   # (512, 512) = Wq.T @ Wk  (d, d')
    WvT = nc.tensor_by_name["WvT"].ap()
    W1T = nc.tensor_by_name["W1T"].ap()     # (512, 512) = (W1*ln_g).T  (e, f)
    W2T = nc.tensor_by_name["W2T"].ap()
    b2 = nc.tensor_by_name["b2"].ap()
    r1 = nc.tensor_by_name["r1"].ap()       # (512,) = W1 @ ln_g
    w1bb1 = nc.tensor_by_name["w1bb1"].ap()  # (512,) = W1 @ ln_b + b1
    outT = nc.tensor_by_name["outT"].ap()   # (512, 1024) fp32 out

    # ---------------- SBUF tiles ----------------
    from contextlib import ExitStack
    ctx = ExitStack()
    consts = ctx.enter_context(tc.tile_pool(name="consts", bufs=1))
    big = ctx.enter_context(tc.tile_pool(name="big", bufs=1))
    qt_pool = ctx.enter_context(tc.tile_pool(name="qt", bufs=2))
    exp_pool = ctx.enter_context(tc.tile_pool(name="expp", bufs=2))
    res_pool = ctx.enter_context(tc.tile_pool(name="resp", bufs=2))
    h1_pool = ctx.enter_context(tc.tile_pool(name="h1p", bufs=2))
    out_pool = ctx.enter_context(tc.tile_pool(name="outp", bufs=2))
    sq_pool = ctx.enter_context(tc.tile_pool(name="sqp", bufs=4))
    row_pool = ctx.enter_context(tc.tile_pool(name="rowp", bufs=2))
    bc_pool = ctx.enter_context(tc.tile_pool(name="bcp", bufs=2))

    mm_psum = ctx.enter_context(tc.tile_pool(name="mmps", bufs=8, space="PSUM"))

    # constants / weights (W1T is pre-scaled by ln_g on the host; r1 and
    # w1bb1 = W1@ln_b + b1 are precomputed host-side as well)
    # A arrives ec-major (host-repacked): stationary chunk = a_sb[:, ec, dc, :]
    a_sb = consts.tile([128, ND, ND, 128], BF16)  # (p, ec, dc, e)
    wv_sb = consts.tile([128, ND, D], BF16)
    w1_sb = consts.tile([128, ND, D], BF16)
    w2_sb = consts.tile([128, ND, D], BF16)
    b2_sb = consts.tile([128, ND], F32)
    r1row = consts.tile([1, D], F32R)   # r1 on one partition (rank-1 stationary)
    w1bb1_sb = consts.tile([128, ND], F32)
    # Input DMAs in need-order, spread over the three DMA queues
    # (sync / scalar / gpsimd) so transfers overlap and the GT GEMM can
    # start as early as possible.
    # Wave 1: the GT GEMM consumes (a_dc, x_dc) pairs in dc order at ~1us
    # cadence, and both GT s-blocks read the full query half [0:1024).
    # 2KB-per-partition runs halve the DMA packet count vs 1KB.
    x_sb = big.tile([128, ND, S], BF16, tag="x", name="x_sb")
    ar = A_qk.rearrange("(ec p) (dc e) -> p ec dc e", p=128, e=128)
    xr = xT.rearrange("(dc p) t -> p dc t", p=128)
    xtm_sb = big.tile([128, NT, D], BF16, tag="v", name="xtm_sb")
    xmr = xTM.rearrange("(tc p) d -> p tc d", p=128)
    wvr = WvT.rearrange("(dc p) e -> p dc e", p=128)
    wr1 = W1T.rearrange("(dc p) e -> p dc e", p=128)
    wr2 = W2T.rearrange("(dc p) e -> p dc e", p=128)

    def xq(dc, h):  # quarter-tile of x: 128KB, 1KB runs
        return (x_sb[:, dc, h * 512:(h + 1) * 512],
                xr[:, dc, h * 512:(h + 1) * 512])

    def xo(dc):  # other-half tile of x: 256KB, 2KB runs
        return (x_sb[:, dc, 1024:2048], xr[:, dc, 1024:2048])

    def xtm(g):
        return (xtm_sb[:, 4 * g:4 * (g + 1), :], xmr[:, 4 * g:4 * (g + 1), :])

    # Per-queue issue order matches first-use order on the PE:
    # GT sb0 consumes a ec-group 0 + x[0:512] chunks first, GT sb1
    # x[512:1024], the scores GEMM then reads x[:,1024:2048], then xtm etc.
    def aq(ec):
        return (a_sb[:, ec, :, :], ar[:, ec, :, :])

    sync_q = [xq(0, 0), xq(2, 0), aq(2), xq(0, 1), xq(2, 1),
              xo(0), xtm(0), xtm(3)]
    scalar_q = [aq(0), aq(1), xo(1), xtm(1), (wv_sb[:, :, :], wvr[:, :, :]),
                (w2_sb[:, :, :], wr2[:, :, :])]
    gpsimd_q = [xq(1, 0), xq(3, 0), aq(3), xq(1, 1), xq(3, 1),
                xo(2), xo(3), xtm(2)]
    gpsimd_q += [(v_sb[:, :], v_dram.rearrange("(c p) -> p c", p=128))
                 for v_sb, v_dram in ((b2_sb, b2), (w1bb1_sb, w1bb1))]
    gpsimd_q.append((r1row[:, :],
                     r1.bitcast(F32R).rearrange("(c e) -> c e", c=1)))
    gpsimd_q.append((w1_sb[:, :, :], wr1[:, :, :]))
    for eng, q in ((nc.sync, sync_q), (nc.scalar, scalar_q),
                   (nc.gpsimd, gpsimd_q)):
        for dst, src in q:
            eng.dma_start(out=dst, in_=src)

    ones128 = nc.tensor_by_name["ones128"].ap()  # (128,) of 1.0
    ones_col_b = consts.tile([128, 1], BF16)   # stationary for column sums
    nc.vector.memset(ones_col_b, 1.0)
    ones_row = consts.tile([1, 128], F32R)      # stationary for partition broadcast
    nc.gpsimd.dma_start(out=ones_row[:, :],
                        in_=ones128.bitcast(F32R).rearrange("(c p) -> c p", c=1))

    for _iter in range(n_iters):
        _emit_iter(nc, tc, x_sb, xtm_sb, outT, big, qt_pool, exp_pool, res_pool,
                   h1_pool, out_pool, sq_pool, row_pool, bc_pool, mm_psum,
                   a_sb, wv_sb, w1_sb, w2_sb, b2_sb,
                   ones_col_b, ones_row, r1row, w1bb1_sb)

    ctx.close()


def _emit_iter(nc, tc, x_sb, xtm_sb, outT, big, qt_pool, exp_pool, res_pool,
               h1_pool, out_pool, sq_pool, row_pool, bc_pool, mm_psum,
               a_sb, wv_sb, w1_sb, w2_sb, b2_sb,
               ones_col_b, ones_row, r1row, w1bb1_sb):
    # ------- GT = A-stationary GEMM (G = x @ A; scores = G @ x^T) -------
    qt_tiles = []
    for sb in range(NBLK):
        s0 = sb * SBLK
        qt_sb = qt_pool.tile([128, ND, SBLK], BF16, tag="qt")
        for ec in range(ND):
            qps = mm_psum.tile([128, 512], F32, tag="mm")
            for dc in range(ND):
                nc.tensor.matmul(
                    qps[:, :],
                    a_sb[:, ec, dc, :],
                    x_sb[:, dc, s0:s0 + SBLK],
                    start=(dc == 0), stop=(dc == ND - 1),
                )
            nc.scalar.copy(out=qt_sb[:, ec, :], in_=qps[:, :])
        qt_tiles.append(qt_sb)

    # ---------------- per s-block pipeline (software-pipelined emission) ----
    # emission order: scores(0), res(0), scores(1), stats(0), res(1),
    # norm+mlp(0), stats(1), norm+mlp(1) - keeps matmul work queued on PE
    # while DVE/ACT compute the LN row stats of the previous block.
    exp_tiles = [None] * NBLK
    esum_tiles = [None] * NBLK
    res_tiles = [None] * NBLK
    rows2_tiles = [None] * NBLK

    def emit_scores(sb):
        qt_sb = qt_tiles[sb]
        exp_sb = exp_pool.tile([128, NT, SBLK], BF16, tag="exp", name=f"exp{sb}")
        for tc_i in range(NT):
            sps = mm_psum.tile([128, 512], F32, tag="mm")
            for dc in range(ND):
                nc.tensor.matmul(
                    sps[:, :],
                    x_sb[:, dc, tc_i * 128:(tc_i + 1) * 128],
                    qt_sb[:, dc, :],
                    start=(dc == 0), stop=(dc == ND - 1),
                )
            nc.scalar.activation(out=exp_sb[:, tc_i, :], in_=sps[:, :],
                                 func=AF.Exp, scale=SCALE)
        exp_tiles[sb] = exp_sb
        # DVE pairwise-add tree over the 16 t-chunks: trails the exp ACTs
        # while PE streams the scores GEMM, so the softmax-denominator
        # reduction needs a single ones-matmul instead of 16.
        es = exp_pool.tile([128, 8, SBLK], BF16, tag="es", name=f"es{sb}")
        for j in range(8):
            nc.vector.tensor_add(out=es[:, j, :], in0=exp_sb[:, 2 * j, :],
                                 in1=exp_sb[:, 2 * j + 1, :])
        for lvl in (4, 2, 1):
            for j in range(lvl):
                nc.vector.tensor_add(out=es[:, j, :], in0=es[:, 2 * j, :],
                                     in1=es[:, 2 * j + 1, :])
        esum_tiles[sb] = es

    def emit_res(sb):
        exp_sb = exp_tiles[sb]
        # Z[d, s] = sum_t x[t,d] * exp[t,s]   (x t-major stationary)
        z_sb = sq_pool.tile([128, ND, SBLK], BF16, tag="z", name=f"z{sb}")
        for dc in range(ND):
            zps = mm_psum.tile([128, 512], F32, tag="mm")
            for tc_i in range(NT):
                nc.tensor.matmul(
                    zps[:, :],
                    xtm_sb[:, tc_i, dc * 128:(dc + 1) * 128],
                    exp_sb[:, tc_i, :],
                    start=(tc_i == 0), stop=(tc_i == NT - 1),
                )
            nc.scalar.copy(out=z_sb[:, dc, :], in_=zps[:, :])
        # resU[e, s] = Wv @ Z
        res_sb = res_pool.tile([128, ND, SBLK], BF16, tag="res", name=f"res{sb}")
        for ec in range(ND):
            rps = mm_psum.tile([128, 512], F32, tag="mm")
            for dc in range(ND):
                nc.tensor.matmul(
                    rps[:, :],
                    wv_sb[:, dc, ec * 128:(ec + 1) * 128],
                    z_sb[:, dc, :],
                    start=(dc == 0), stop=(dc == ND - 1),
                )
            nc.scalar.copy(out=res_sb[:, ec, :], in_=rps[:, :])
        res_tiles[sb] = res_sb

    rt_tiles = [None] * NBLK
    sq_tiles = [None] * NBLK

    def emit_stats_dve(sb):
        # DVE add-trees over the 4 e-chunks for sum(res) and sum(res^2)
        res_sb = res_tiles[sb]
        rt = sq_pool.tile([128, 2, SBLK], BF16, tag="rt", name=f"rt{sb}")
        nc.vector.tensor_add(out=rt[:, 0, :], in0=res_sb[:, 0, :],
                             in1=res_sb[:, 1, :])
        nc.vector.tensor_add(out=rt[:, 1, :], in0=res_sb[:, 2, :],
                             in1=res_sb[:, 3, :])
        nc.vector.tensor_add(out=rt[:, 0, :], in0=rt[:, 0, :], in1=rt[:, 1, :])
        # squares on ACT (has slack here) to unload the DVE; adds stay DVE
        sq_sb = sq_pool.tile([128, ND, SBLK], BF16, tag="sq", name=f"sq{sb}")
        for ec in range(ND):
            nc.scalar.square(out=sq_sb[:, ec, :], in_=res_sb[:, ec, :])
        nc.vector.tensor_add(out=sq_sb[:, 0, :], in0=sq_sb[:, 0, :],
                             in1=sq_sb[:, 1, :])
        nc.vector.tensor_add(out=sq_sb[:, 2, :], in0=sq_sb[:, 2, :],
                             in1=sq_sb[:, 3, :])
        nc.vector.tensor_add(out=sq_sb[:, 0, :], in0=sq_sb[:, 0, :],
                             in1=sq_sb[:, 2, :])
        rt_tiles[sb] = rt
        sq_tiles[sb] = sq_sb

    def emit_stats(sb):
        sums_ps = mm_psum.tile([1, 512], F32, tag="mm")
        nc.tensor.matmul(sums_ps[:, :], ones_col_b[:, :],
                         esum_tiles[sb][:, 0, :], start=True, stop=True)
        sume_ps = mm_psum.tile([1, 512], F32, tag="mm")
        nc.tensor.matmul(sume_ps[:, :], ones_col_b[:, :],
                         rt_tiles[sb][:, 0, :], start=True, stop=True)
        sumsq_ps = mm_psum.tile([1, 512], F32, tag="mm")
        nc.tensor.matmul(sumsq_ps[:, :], ones_col_b[:, :],
                         sq_tiles[sb][:, 0, :], start=True, stop=True)

        # row stats on one partition (all on DVE where [1,512] ops are ~140ns
        # vs ~680ns on ACT; only sqrt stays on ACT):
        #   muU = sumE/512 ; varU = sumSq/512 - muU^2
        #   rstd = 1/sqrt(varU + EPS*sums^2) ; murstd = muU*rstd
        rows = row_pool.tile([1, 4, SBLK], F32, tag="rows", name=f"rows{sb}")
        rows2 = row_pool.tile([1, 2, SBLK], F32R, tag="rows2", name=f"rows2{sb}")
        nc.vector.tensor_scalar_mul(out=rows[:, 0, :], in0=sume_ps[:, :],
                                    scalar1=-1.0 / D)                        # -muU
        # -muU feeds the P GEMM's rank-1 accumulation: publish it first so
        # the PE doesn't wait on the sqrt/reciprocal part of this chain
        nc.vector.tensor_copy(out=rows2[:, 1, :], in_=rows[:, 0, :])         # -muU
        nc.scalar.activation(out=rows[:, 2, :], in_=sums_ps[:, :],
                             func=AF.Square, scale=float(np.sqrt(EPS)))      # eps*sums^2
        nc.vector.tensor_mul(out=rows[:, 3, :], in0=rows[:, 0, :], in1=rows[:, 0, :])
        nc.vector.scalar_tensor_tensor(
            out=rows[:, 1, :], in0=sumsq_ps[:, :], scalar=1.0 / D,
            in1=rows[:, 3, :], op0=ALU.mult, op1=ALU.subtract)               # msq-muU^2
        nc.vector.tensor_add(out=rows[:, 1, :], in0=rows[:, 1, :], in1=rows[:, 2, :])
        nc.scalar.activation(out=rows[:, 1, :], in_=rows[:, 1, :], func=AF.Sqrt)
        nc.vector.reciprocal_approx_fast(out=rows[:, 3, :], in_=rows[:, 1, :])
        nc.vector.tensor_copy(out=rows2[:, 0, :], in_=rows[:, 3, :])         # rstd
        rows2_tiles[sb] = rows2

    p_tiles = [None] * NBLK
    h1_tiles = [None] * NBLK

    def emit_p(sb):
        # P = G1 @ res, plus a rank-1 matmul accumulating murstd[s]*r1[f]
        # into the same PSUM (replaces a per-fc DVE scalar_tensor_tensor in
        # the epilogue; the DVE queue is the end-phase bottleneck).
        res_sb = res_tiles[sb]
        rows2 = rows2_tiles[sb]
        p_ps = []
        for fc in range(ND):
            hps = mm_psum.tile([128, 512], F32, tag="mm", name=f"p{sb}_{fc}")
            for ec in range(ND):
                nc.tensor.matmul(
                    hps[:, :],
                    w1_sb[:, ec, fc * 128:(fc + 1) * 128],
                    res_sb[:, ec, :],
                    start=(ec == 0), stop=False,
                )
            nc.tensor.matmul(
                hps[:, :], r1row[:, fc * 128:(fc + 1) * 128],
                rows2[:, 1, :], start=False, stop=True,
            )
            p_ps.append(hps)
        p_tiles[sb] = p_ps

    def emit_bc_epi(sb):
        rows2 = rows2_tiles[sb]
        p_ps = p_tiles[sb]

        # broadcast rstd across 128 partitions via a K=1 matmul
        bc_sb = bc_pool.tile([128, SBLK], F32, tag="bc_sb")
        bc_ps = mm_psum.tile([128, 512], F32, tag="mm")
        nc.tensor.matmul(
            bc_ps[:, :], ones_row[:, :],
            rows2[:, 0, :], start=True, stop=True,
        )
        nc.scalar.copy(out=bc_sb[:, :], in_=bc_ps[:, :])

        # fused MLP1 + LayerNorm epilogue (murstd*r1 already accumulated
        # into the P psum by emit_p):
        #   h1 = relu( (P - murstd[s]*r1[f])*rstd[s] ... wait, P already
        #   holds G1@res + murstd*r1; h1 = relu(P*rstd + w1b[f] + b1[f])
        h1_sb = h1_pool.tile([128, ND, SBLK], BF16, tag="h1", name=f"h1_{sb}")
        for fc in range(ND):
            t_sb = sq_pool.tile([128, SBLK], F32R, tag="sq")
            nc.vector.tensor_mul(out=t_sb[:, :], in0=p_ps[fc][:, :],
                                 in1=bc_sb[:, :])
            nc.scalar.activation(out=h1_sb[:, fc, :], in_=t_sb[:, :],
                                 func=AF.Relu, bias=w1bb1_sb[:, fc:fc + 1])
        h1_tiles[sb] = h1_sb

    def emit_mlp2(sb):
        s0 = sb * SBLK
        h1_sb = h1_tiles[sb]
        o_sb = out_pool.tile([128, ND, SBLK], BF16, tag="o")
        outr = outT[:, s0:s0 + SBLK].rearrange("(gc p) s -> p gc s", p=128)
        for gc in range(ND):
            ops = mm_psum.tile([128, 512], F32, tag="mm")
            for fc in range(ND):
                nc.tensor.matmul(
                    ops[:, :],
                    w2_sb[:, fc, gc * 128:(gc + 1) * 128],
                    h1_sb[:, fc, :],
                    start=(fc == 0), stop=(fc == ND - 1),
                )
            # alternate the bias epilogue between ACT and DVE so the final
            # chunks drain in parallel instead of serializing on one engine
            if gc % 2 == 0:
                nc.scalar.activation(out=o_sb[:, gc, :], in_=ops[:, :],
                                     func=AF.Identity, bias=b2_sb[:, gc:gc + 1])
            else:
                nc.vector.tensor_scalar_add(out=o_sb[:, gc, :], in0=ops[:, :],
                                            scalar1=b2_sb[:, gc:gc + 1])
            eng = (nc.sync, nc.scalar, nc.gpsimd, nc.sync)[gc]
            eng.dma_start(out=outr[:, gc, :], in_=o_sb[:, gc, :])

    emit_scores(0)
    emit_res(0)
    emit_stats_dve(0)
    emit_scores(1)
    emit_stats(0)
    emit_res(1)
    emit_stats_dve(1)
    emit_p(0)
    emit_stats(1)
    emit_bc_epi(0)
    emit_p(1)
    emit_bc_epi(1)
    emit_mlp2(0)
    emit_mlp2(1)


def build_nc(n_iters=1):
    nc = bacc.Bacc("TRN2", target_bir_lowering=False, debug=False)
    nc.tensor_by_name = {}

    def dram(name, shape, kind):
        t = nc.dram_tensor(name, shape, F32, kind=kind)
        nc.tensor_by_name[name] = t
        return t

    def dram_bf(name, shape, kind):
        t = nc.dram_tensor(name, shape, BF16, kind=kind)
        nc.tensor_by_name[name] = t
        return t

    dram_bf("xT", [D, S], "ExternalInput")
    dram_bf("xTM", [S, D], "ExternalInput")
    for nm in ("A_qk", "WvT", "W1T", "W2T"):
        dram_bf(nm, [D, D], "ExternalInput")
    for nm in ("b2", "r1", "w1bb1"):
        dram(nm, [D], "ExternalInput")
    dram("ones128", [128], "ExternalInput")
    dram_bf("outT", [D, SQ], "ExternalOutput")

    with tile.TileContext(nc) as tc:
        _emit(nc, tc, n_iters=n_iters)
    nc.compile()
    return nc


_CACHED_NC = None


def _get_nc():
    global _CACHED_NC
    if _CACHED_NC is None:
        _CACHED_NC = build_nc()
    return _CACHED_NC


def make_in_maps(x, Wq, Wk, Wv, ln_g, ln_b, W1, b1, W2, b2):
    BF = ml_dtypes.bfloat16
    x = np.asarray(x, dtype=np.float32)
    A_qk = np.asarray(Wq, np.float32).T @ np.asarray(Wk, np.float32)
    # repack ec-major: A2[ec*128+p, dc*128+e] = A_qk[dc*128+p, ec*128+e]
    A_qk = np.ascontiguousarray(
        A_qk.reshape(4, 128, 4, 128).transpose(2, 1, 0, 3).reshape(512, 512))
    W1f = np.asarray(W1, np.float32)
    gf = np.asarray(ln_g, np.float32)
    # LayerNorm fold, precomputed host-side:
    #   G1 = W1 * ln_g[e]  (pre-scaled MLP1 weight)
    #   r1 = W1 @ ln_g     (rank-1 -mu*rstd correction row)
    #   w1bb1 = W1 @ ln_b + b1
    shared = {
        "A_qk": np.ascontiguousarray(A_qk.astype(BF)),
        "WvT": np.ascontiguousarray(np.asarray(Wv, np.float32).T.astype(BF)),
        "W1T": np.ascontiguousarray((W1f * gf[None, :]).T.astype(BF)),
        "W2T": np.ascontiguousarray(np.asarray(W2, np.float32).T.astype(BF)),
        "b2": np.asarray(b2, np.float32),
        "r1": W1f @ gf,
        "w1bb1": W1f @ np.asarray(ln_b, np.float32) + np.asarray(b1, np.float32),
        "ones128": np.ones(128, np.float32),
    }
    in_maps = []
    for c in range(N_CORES):
        b, h = divmod(c, 2)
        xT = x[:, b, :].T  # (512, 2048)
        q = xT[:, h * SQ:(h + 1) * SQ]
        o = xT[:, (1 - h) * SQ:(2 - h) * SQ]
        xp = np.concatenate([q, o], axis=1)  # (512, 2048), q-half first
        in_maps.append({"xT": np.ascontiguousarray(xp.astype(BF)),
                        "xTM": np.ascontiguousarray(xp.T.astype(BF)),
                        **shared})
    return in_maps


def kernel(x, Wq, Wk, Wv, ln_g, ln_b, W1, b1, W2, b2):
    nc = _get_nc()
    in_maps = make_in_maps(x, Wq, Wk, Wv, ln_g, ln_b, W1, b1, W2, b2)
    res = run_bass_kernel_spmd(nc, in_maps, list(range(N_CORES)))
    out = np.empty((S, B, D), dtype=np.float32)
    for c in range(N_CORES):
        b, h = divmod(c, 2)
        out[h * SQ:(h + 1) * SQ, b, :] = res.results[c]["outT"].T.astype(np.float32)
    return out



# revision 60
# speedup vs baseline: 1.0053x; 1.0053x over previous
"""Trainium2 Bass kernel for the attention+LN+MLP block (nn_Attention_84310208020626).

Reference computation (per batch b):
    q = x_b @ Wq.T ; k = x_b @ Wk.T ; v = x_b @ Wv.T          (S=2048, D=512)
    attn = softmax(q k^T / sqrt(512))
    res  = attn @ v
    h    = LayerNorm(res) * ln_g + ln_b
    out  = relu(h @ W1.T + b1) @ W2.T + b2

Sharding: 8 cores = 4 batches x 2 sequence halves. Every core computes its
batch's full K/V (recompute, no collectives) and runs attention + LN + MLP
for its own 1024 query rows.

Device layout: activations are feature-major [feature, seq] so that every
GEMM contracts over the partition dimension without transposes:
    GT[d',s]     = A-stationary GEMM over xT, A = Wq^T Wk precomputed on host
                   (scores = q k^T = (x A) x^T, so no separate Q/K GEMMs)
    scoresT[t,s] = xT-stationary GEMM, rhs = GT     -> exp -> expT (bf16)
    Z[d,s]       = xTM-stationary GEMM over expT; resU[e,s] = WvT @ Z
LayerNorm sums over e and the softmax denominator use DVE pairwise-add
trees followed by a single ones-stationary matmul each; the softmax
division is folded into LN via scale invariance with a corrected epsilon:
    LN(res) = (resU - muU) / sqrt(varU + eps*sums^2)  (exact in exact arithmetic)
and the whole LN is folded into the MLP1 GEMM epilogue:
    h1 = relu( (G1 @ res - muU[s]*r1[f])*rstd[s] + (W1@ln_b)[f] + b1[f] )
where the rank-1 term -muU[s]*r1[f] is accumulated into the P PSUM by a
K=1 matmul (stationary = r1 row) and rstd is broadcast across partitions
with a K=1 ones matmul. G1 = W1*diag(ln_g), r1 = W1 @ ln_g and
w1bb1 = W1 @ ln_b + b1 are precomputed on the host (like A_qk).
All GEMM operands are bf16 (fp32 PSUM accumulation); LN stats math is fp32
with a fast-approximate reciprocal (~18 bits) for 1/sqrt(var).
Input DMAs are staged in first-use order across the three DMA queues
(sync/scalar/gpsimd); A is repacked ec-major on the host so the GT GEMM's
first stationary group is the first 128KB off the wire.
"""

import ml_dtypes
import numpy as np

import concourse.bass as bass
import concourse.mybir as mybir
import concourse.tile as tile
from concourse import bacc
from concourse.bass_utils import run_bass_kernel_spmd

S, B, D = 2048, 4, 512
N_CORES = 8
SQ = 1024          # query rows per core
SBLK = 512         # s-block (pipeline granularity)
NBLK = SQ // SBLK  # 2
ND = D // 128      # 4 chunks of the feature dims
NT = S // 128      # 16 t-chunks
NTT = S // 512     # 4 t-tiles of 512 for KT GEMM
EPS = 1e-5
SCALE = 1.0 / float(np.sqrt(512.0))

F32 = mybir.dt.float32
F32R = mybir.dt.float32r
BF16 = mybir.dt.bfloat16
AF = mybir.ActivationFunctionType
ALU = mybir.AluOpType


def _emit(nc, tc, n_iters=1):
    xT = nc.tensor_by_name["xT"].ap()       # (512, 2048) bf16, q-half first
    xTM = nc.tensor_by_name["xTM"].ap()     # (2048, 512) bf16, same t order
    A_qk = nc.tensor_by_name["A_qk"].ap()   # (512, 512) = Wq.T @ Wk  (d, d')
    WvT = nc.tensor_by_name["WvT"].ap()
    W1T = nc.tensor_by_name["W1T"].ap()     # (512, 512) = (W1*ln_g).T  (e, f)
    W2T = nc.tensor_by_name["W2T"].ap()
    b2 = nc.tensor_by_name["b2"].ap()
    r1 = nc.tensor_by_name["r1"].ap()       # (512,) = W1 @ ln_g
    w1bb1 = nc.tensor_by_name["w1bb1"].ap()  # (512,) = W1 @ ln_b + b1
    outT = nc.tensor_by_name["outT"].ap()   # (512, 1024) fp32 out

    # ---------------- SBUF tiles ----------------
    from contextlib import ExitStack
    ctx = ExitStack()
    consts = ctx.enter_context(tc.tile_pool(name="consts", bufs=1))
    big = ctx.enter_context(tc.tile_pool(name="big", bufs=1))
    qt_pool = ctx.enter_context(tc.tile_pool(name="qt", bufs=2))
    exp_pool = ctx.enter_context(tc.tile_pool(name="expp", bufs=2))
    res_pool = ctx.enter_context(tc.tile_pool(name="resp", bufs=2))
    h1_pool = ctx.enter_context(tc.tile_pool(name="h1p", bufs=2))
    out_pool = ctx.enter_context(tc.tile_pool(name="outp", bufs=2))
    sq_pool = ctx.enter_context(tc.tile_pool(name="sqp", bufs=4))
    row_pool = ctx.enter_context(tc.tile_pool(name="rowp", bufs=2))
    bc_pool = ctx.enter_context(tc.tile_pool(name="bcp", bufs=2))

    mm_psum = ctx.enter_context(tc.tile_pool(name="mmps", bufs=8, space="PSUM"))

    # constants / weights (W1T is pre-scaled by ln_g on the host; r1 and
    # w1bb1 = W1@ln_b + b1 are precomputed host-side as well)
    # A arrives ec-major (host-repacked): stationary chunk = a_sb[:, ec, dc, :]
    a_sb = consts.tile([128, ND, ND, 128], BF16)  # (p, ec, dc, e)
    wv_sb = consts.tile([128, ND, D], BF16)
    w1_sb = consts.tile([128, ND, D], BF16)
    w2_sb = consts.tile([128, ND, D], BF16)
    b2_sb = consts.tile([128, ND], F32)
    r1row = consts.tile([1, D], F32R)   # r1 on one partition (rank-1 stationary)
    w1bb1_sb = consts.tile([128, ND], F32)
    # Input DMAs in need-order, spread over the three DMA queues
    # (sync / scalar / gpsimd) so transfers overlap and the GT GEMM can
    # start as early as possible.
    # Wave 1: the GT GEMM consumes (a_dc, x_dc) pairs in dc order at ~1us
    # cadence, and both GT s-blocks read the full query half [0:1024).
    # 2KB-per-partition runs halve the DMA packet count vs 1KB.
    x_sb = big.tile([128, ND, S], BF16, tag="x", name="x_sb")
    ar = A_qk.rearrange("(ec p) (dc e) -> p ec dc e", p=128, e=128)
    xr = xT.rearrange("(dc p) t -> p dc t", p=128)
    xtm_sb = big.tile([128, NT, D], BF16, tag="v", name="xtm_sb")
    xmr = xTM.rearrange("(tc p) d -> p tc d", p=128)
    wvr = WvT.rearrange("(dc p) e -> p dc e", p=128)
    wr1 = W1T.rearrange("(dc p) e -> p dc e", p=128)
    wr2 = W2T.rearrange("(dc p) e -> p dc e", p=128)

    def xq(dc, h):  # quarter-tile of x: 128KB, 1KB runs
        return (x_sb[:, dc, h * 512:(h + 1) * 512],
                xr[:, dc, h * 512:(h + 1) * 512])

    def xo(dc):  # other-half tile of x: 256KB, 2KB runs
        return (x_sb[:, dc, 1024:2048], xr[:, dc, 1024:2048])

    def xtm(g):
        return (xtm_sb[:, 4 * g:4 * (g + 1), :], xmr[:, 4 * g:4 * (g + 1), :])

    # Per-queue issue order matches first-use order on the PE:
    # GT sb0 consumes a ec-group 0 + x[0:512] chunks first, GT sb1
    # x[512:1024], the scores GEMM then reads x[:,1024:2048], then xtm etc.
    def aq(ec):
        return (a_sb[:, ec, :, :], ar[:, ec, :, :])

    sync_q = [xq(0, 0), xq(2, 0), aq(2), xq(0, 1), xq(2, 1),
              xo(0), xtm(0), xtm(3)]
    scalar_q = [aq(0), aq(1), xo(1), xtm(1), (wv_sb[:, :, :], wvr[:, :, :]),
                (w2_sb[:, :, :], wr2[:, :, :])]
    gpsimd_q = [xq(1, 0), xq(3, 0), aq(3), xq(1, 1), xq(3, 1),
                xo(2), xo(3), xtm(2)]
    gpsimd_q += [(v_sb[:, :], v_dram.rearrange("(c p) -> p c", p=128))
                 for v_sb, v_dram in ((b2_sb, b2), (w1bb1_sb, w1bb1))]
    gpsimd_q.append((r1row[:, :],
                     r1.bitcast(F32R).rearrange("(c e) -> c e", c=1)))
    gpsimd_q.append((w1_sb[:, :, :], wr1[:, :, :]))
    for eng, q in ((nc.sync, sync_q), (nc.scalar, scalar_q),
                   (nc.gpsimd, gpsimd_q)):
        for dst, src in q:
            eng.dma_start(out=dst, in_=src)

    ones128 = nc.tensor_by_name["ones128"].ap()  # (128,) of 1.0
    ones_col_b = consts.tile([128, 1], BF16)   # stationary for column sums
    nc.vector.memset(ones_col_b, 1.0)
    ones_row = consts.tile([1, 128], F32R)      # stationary for partition broadcast
    nc.gpsimd.dma_start(out=ones_row[:, :],
                        in_=ones128.bitcast(F32R).rearrange("(c p) -> c p", c=1))

    for _iter in range(n_iters):
        _emit_iter(nc, tc, x_sb, xtm_sb, outT, big, qt_pool, exp_pool, res_pool,
                   h1_pool, out_pool, sq_pool, row_pool, bc_pool, mm_psum,
                   a_sb, wv_sb, w1_sb, w2_sb, b2_sb,
                   ones_col_b, ones_row, r1row, w1bb1_sb)

    ctx.close()


def _emit_iter(nc, tc, x_sb, xtm_sb, outT, big, qt_pool, exp_pool, res_pool,
               h1_pool, out_pool, sq_pool, row_pool, bc_pool, mm_psum,
               a_sb, wv_sb, w1_sb, w2_sb, b2_sb,
               ones_col_b, ones_row, r1row, w1bb1_sb):
    # ------- GT = A-stationary GEMM (G = x @ A; scores = G @ x^T) -------
    qt_tiles = []
    for sb in range(NBLK):
        s0 = sb * SBLK
        qt_sb = qt_pool.tile([128, ND, SBLK], BF16, tag="qt")
        for ec in range(ND):
            qps = mm_psum.tile([128, 512], F32, tag="mm")
            for dc in range(ND):
                nc.tensor.matmul(
                    qps[:, :],
                    a_sb[:, ec, dc, :],
                    x_sb[:, dc, s0:s0 + SBLK],
                    start=(dc == 0), stop=(dc == ND - 1),
                )
            nc.scalar.copy(out=qt_sb[:, ec, :], in_=qps[:, :])
        qt_tiles.append(qt_sb)

    # ---------------- per s-block pipeline (software-pipelined emission) ----
    # emission order: scores(0), res(0), scores(1), stats(0), res(1),
    # norm+mlp(0), stats(1), norm+mlp(1) - keeps matmul work queued on PE
    # while DVE/ACT compute the LN row stats of the previous block.
    exp_tiles = [None] * NBLK
    esum_tiles = [None] * NBLK
    res_tiles = [None] * NBLK
    rows2_tiles = [None] * NBLK

    def emit_scores(sb):
        qt_sb = qt_tiles[sb]
        exp_sb = exp_pool.tile([128, NT, SBLK], BF16, tag="exp", name=f"exp{sb}")
        for tc_i in range(NT):
            sps = mm_psum.tile([128, 512], F32, tag="mm")
            for dc in range(ND):
                nc.tensor.matmul(
                    sps[:, :],
                    x_sb[:, dc, tc_i * 128:(tc_i + 1) * 128],
                    qt_sb[:, dc, :],
                    start=(dc == 0), stop=(dc == ND - 1),
                )
            nc.scalar.activation(out=exp_sb[:, tc_i, :], in_=sps[:, :],
                                 func=AF.Exp, scale=SCALE)
        exp_tiles[sb] = exp_sb
        # DVE pairwise-add tree over the 16 t-chunks: trails the exp ACTs
        # while PE streams the scores GEMM, so the softmax-denominator
        # reduction needs a single ones-matmul instead of 16.
        es = exp_pool.tile([128, 8, SBLK], BF16, tag="es", name=f"es{sb}")
        for j in range(8):
            nc.vector.tensor_add(out=es[:, j, :], in0=exp_sb[:, 2 * j, :],
                                 in1=exp_sb[:, 2 * j + 1, :])
        for lvl in (4, 2, 1):
            for j in range(lvl):
                nc.vector.tensor_add(out=es[:, j, :], in0=es[:, 2 * j, :],
                                     in1=es[:, 2 * j + 1, :])
        esum_tiles[sb] = es

    def emit_res(sb):
        exp_sb = exp_tiles[sb]
        # Z[d, s] = sum_t x[t,d] * exp[t,s]   (x t-major stationary)
        z_sb = sq_pool.tile([128, ND, SBLK], BF16, tag="z", name=f"z{sb}")
        for dc in range(ND):
            zps = mm_psum.tile([128, 512], F32, tag="mm")
            for tc_i in range(NT):
                nc.tensor.matmul(
                    zps[:, :],
                    xtm_sb[:, tc_i, dc * 128:(dc + 1) * 128],
                    exp_sb[:, tc_i, :],
                    start=(tc_i == 0), stop=(tc_i == NT - 1),
                )
            nc.scalar.copy(out=z_sb[:, dc, :], in_=zps[:, :])
        # resU[e, s] = Wv @ Z
        res_sb = res_pool.tile([128, ND, SBLK], BF16, tag="res", name=f"res{sb}")
        for ec in range(ND):
            rps = mm_psum.tile([128, 512], F32, tag="mm")
            for dc in range(ND):
                nc.tensor.matmul(
                    rps[:, :],
                    wv_sb[:, dc, ec * 128:(ec + 1) * 128],
                    z_sb[:, dc, :],
                    start=(dc == 0), stop=(dc == ND - 1),
                )
            nc.scalar.copy(out=res_sb[:, ec, :], in_=rps[:, :])
        res_tiles[sb] = res_sb

    rt_tiles = [None] * NBLK
    sq_tiles = [None] * NBLK

    def emit_stats_dve(sb):
        # DVE add-trees over the 4 e-chunks for sum(res) and sum(res^2)
        res_sb = res_tiles[sb]
        rt = sq_pool.tile([128, 2, SBLK], BF16, tag="rt", name=f"rt{sb}")
        nc.vector.tensor_add(out=rt[:, 0, :], in0=res_sb[:, 0, :],
                             in1=res_sb[:, 1, :])
        nc.vector.tensor_add(out=rt[:, 1, :], in0=res_sb[:, 2, :],
                             in1=res_sb[:, 3, :])
        nc.vector.tensor_add(out=rt[:, 0, :], in0=rt[:, 0, :], in1=rt[:, 1, :])
        # squares on ACT (has slack here) to unload the DVE; adds stay DVE
        sq_sb = sq_pool.tile([128, ND, SBLK], BF16, tag="sq", name=f"sq{sb}")
        for ec in range(ND):
            nc.scalar.square(out=sq_sb[:, ec, :], in_=res_sb[:, ec, :])
        nc.vector.tensor_add(out=sq_sb[:, 0, :], in0=sq_sb[:, 0, :],
                             in1=sq_sb[:, 1, :])
        nc.vector.tensor_add(out=sq_sb[:, 2, :], in0=sq_sb[:, 2, :],
                             in1=sq_sb[:, 3, :])
        nc.vector.tensor_add(out=sq_sb[:, 0, :], in0=sq_sb[:, 0, :],
                             in1=sq_sb[:, 2, :])
        rt_tiles[sb] = rt
        sq_tiles[sb] = sq_sb

    def emit_stats(sb):
        sums_ps = mm_psum.tile([1, 512], F32, tag="mm")
        nc.tensor.matmul(sums_ps[:, :], ones_col_b[:, :],
                         esum_tiles[sb][:, 0, :], start=True, stop=True)
        sume_ps = mm_psum.tile([1, 512], F32, tag="mm")
        nc.tensor.matmul(sume_ps[:, :], ones_col_b[:, :],
                         rt_tiles[sb][:, 0, :], start=True, stop=True)
        sumsq_ps = mm_psum.tile([1, 512], F32, tag="mm")
        nc.tensor.matmul(sumsq_ps[:, :], ones_col_b[:, :],
                         sq_tiles[sb][:, 0, :], start=True, stop=True)

        # row stats on one partition (all on DVE where [1,512] ops are ~140ns
        # vs ~680ns on ACT; only sqrt stays on ACT):
        #   muU = sumE/512 ; varU = sumSq/512 - muU^2
        #   rstd = 1/sqrt(varU + EPS*sums^2) ; murstd = muU*rstd
        rows = row_pool.tile([1, 4, SBLK], F32, tag="rows", name=f"rows{sb}")
        rows2 = row_pool.tile([1, 2, SBLK], F32R, tag="rows2", name=f"rows2{sb}")
        nc.vector.tensor_scalar_mul(out=rows[:, 0, :], in0=sume_ps[:, :],
                                    scalar1=-1.0 / D)                        # -muU
        # -muU feeds the P GEMM's rank-1 accumulation: publish it first so
        # the PE doesn't wait on the sqrt/reciprocal part of this chain
        nc.vector.tensor_copy(out=rows2[:, 1, :], in_=rows[:, 0, :])         # -muU
        nc.scalar.activation(out=rows[:, 2, :], in_=sums_ps[:, :],
                             func=AF.Square, scale=float(np.sqrt(EPS)))      # eps*sums^2
        nc.vector.tensor_mul(out=rows[:, 3, :], in0=rows[:, 0, :], in1=rows[:, 0, :])
        nc.vector.scalar_tensor_tensor(
            out=rows[:, 1, :], in0=sumsq_ps[:, :], scalar=1.0 / D,
            in1=rows[:, 3, :], op0=ALU.mult, op1=ALU.subtract)               # msq-muU^2
        nc.vector.tensor_add(out=rows[:, 1, :], in0=rows[:, 1, :], in1=rows[:, 2, :])
        nc.scalar.activation(out=rows[:, 1, :], in_=rows[:, 1, :], func=AF.Sqrt)
        nc.vector.reciprocal_approx_fast(out=rows[:, 3, :], in_=rows[:, 1, :])
        nc.vector.tensor_copy(out=rows2[:, 0, :], in_=rows[:, 3, :])         # rstd
        rows2_tiles[sb] = rows2

    p_tiles = [None] * NBLK
    h1_tiles = [None] * NBLK

    def emit_p(sb):
        # P = G1 @ res, plus a rank-1 matmul accumulating murstd[s]*r1[f]
        # into the same PSUM (replaces a per-fc DVE scalar_tensor_tensor in
        # the epilogue; the DVE queue is the end-phase bottleneck).
        res_sb = res_tiles[sb]
        rows2 = rows2_tiles[sb]
        p_ps = []
        for fc in range(ND):
            hps = mm_psum.tile([128, 512], F32, tag="mm", name=f"p{sb}_{fc}")
            for ec in range(ND):
                nc.tensor.matmul(
                    hps[:, :],
                    w1_sb[:, ec, fc * 128:(fc + 1) * 128],
                    res_sb[:, ec, :],
                    start=(ec == 0), stop=False,
                )
            nc.tensor.matmul(
                hps[:, :], r1row[:, fc * 128:(fc + 1) * 128],
                rows2[:, 1, :], start=False, stop=True,
            )
            p_ps.append(hps)
        p_tiles[sb] = p_ps

    def emit_bc_epi(sb):
        rows2 = rows2_tiles[sb]
        p_ps = p_tiles[sb]

        # broadcast rstd across 128 partitions via a K=1 matmul
        bc_sb = bc_pool.tile([128, SBLK], F32, tag="bc_sb")
        bc_ps = mm_psum.tile([128, 512], F32, tag="mm")
        nc.tensor.matmul(
            bc_ps[:, :], ones_row[:, :],
            rows2[:, 0, :], start=True, stop=True,
        )
        nc.scalar.copy(out=bc_sb[:, :], in_=bc_ps[:, :])

        # fused MLP1 + LayerNorm epilogue (murstd*r1 already accumulated
        # into the P psum by emit_p):
        #   h1 = relu( (P - murstd[s]*r1[f])*rstd[s] ... wait, P already
        #   holds G1@res + murstd*r1; h1 = relu(P*rstd + w1b[f] + b1[f])
        h1_sb = h1_pool.tile([128, ND, SBLK], BF16, tag="h1", name=f"h1_{sb}")
        for fc in range(ND):
            t_sb = sq_pool.tile([128, SBLK], F32R, tag="sq")
            nc.vector.tensor_mul(out=t_sb[:, :], in0=p_ps[fc][:, :],
                                 in1=bc_sb[:, :])
            nc.scalar.activation(out=h1_sb[:, fc, :], in_=t_sb[:, :],
                                 func=AF.Relu, bias=w1bb1_sb[:, fc:fc + 1])
        h1_tiles[sb] = h1_sb

    def emit_mlp2(sb):
        s0 = sb * SBLK
        h1_sb = h1_tiles[sb]
        o_sb = out_pool.tile([128, ND, SBLK], BF16, tag="o")
        outr = outT[:, s0:s0 + SBLK].rearrange("(gc p) s -> p gc s", p=128)
        for gc in range(ND):
            ops = mm_psum.tile([128, 512], F32, tag="mm")
            for fc in range(ND):
                nc.tensor.matmul(
                    ops[:, :],
                    w2_sb[:, fc, gc * 128:(gc + 1) * 128],
                    h1_sb[:, fc, :],
                    start=(fc == 0), stop=(fc == ND - 1),
                )
            # alternate the bias epilogue between ACT and DVE so the final
            # chunks drain in parallel instead of serializing on one engine
            if gc % 2 == 0:
                nc.scalar.activation(out=o_sb[:, gc, :], in_=ops[:, :],
                                     func=AF.Identity, bias=b2_sb[:, gc:gc + 1])
            else:
                nc.vector.tensor_scalar_add(out=o_sb[:, gc, :], in0=ops[:, :],
                                            scalar1=b2_sb[:, gc:gc + 1])
            eng = (nc.sync, nc.scalar, nc.gpsimd, nc.sync)[gc]
            eng.dma_start(out=outr[:, gc, :], in_=o_sb[:, gc, :])

    emit_scores(0)
    emit_res(0)
    emit_stats_dve(0)
    emit_scores(1)
    emit_stats(0)
    emit_res(1)
    emit_stats_dve(1)
    emit_p(0)
    emit_stats(1)
    emit_bc_epi(0)
    emit_p(1)
    emit_bc_epi(1)
    emit_mlp2(0)
    emit_mlp2(1)


def build_nc(n_iters=1):
    nc = bacc.Bacc("TRN2", target_bir_lowering=False, debug=False)
    nc.tensor_by_name = {}

    def dram(name, shape, kind):
        t = nc.dram_tensor(name, shape, F32, kind=kind)
        nc.tensor_by_name[name] = t
        return t

    def dram_bf(name, shape, kind):
        t = nc.dram_tensor(name, shape, BF16, kind=kind)
        nc.tensor_by_name[name] = t
        return t

    dram_bf("xT", [D, S], "ExternalInput")
    dram_bf("xTM", [S, D], "ExternalInput")
    for nm in ("A_qk", "WvT", "W1T", "W2T"):
        dram_bf(nm, [D, D], "ExternalInput")
    for nm in ("b2", "r1", "w1bb1"):
        dram(nm, [D], "ExternalInput")
    dram("ones128", [128], "ExternalInput")
    dram_bf("outT", [D, SQ], "ExternalOutput")

    with tile.TileContext(nc) as tc:
        _emit(nc, tc, n_iters=n_iters)
    nc.compile()
    return nc


_CACHED_NC = None


def _get_nc():
    global _CACHED_NC
    if _CACHED_NC is None:
        _CACHED_NC = build_nc()
    return _CACHED_NC


def make_in_maps(x, Wq, Wk, Wv, ln_g, ln_b, W1, b1, W2, b2):
    BF = ml_dtypes.bfloat16
    x = np.asarray(x, dtype=np.float32)
    A_qk = np.asarray(Wq, np.float32).T @ np.asarray(Wk, np.float32)
    # repack ec-major: A2[ec*128+p, dc*128+e] = A_qk[dc*128+p, ec*128+e]
    A_qk = np.ascontiguousarray(
        A_qk.reshape(4, 128, 4, 128).transpose(2, 1, 0, 3).reshape(512, 512))
    W1f = np.asarray(W1, np.float32)
    gf = np.asarray(ln_g, np.float32)
    # LayerNorm fold, precomputed host-side:
    #   G1 = W1 * ln_g[e]  (pre-scaled MLP1 weight)
    #   r1 = W1 @ ln_g     (rank-1 -mu*rstd correction row)
    #   w1bb1 = W1 @ ln_b + b1
    shared = {
        "A_qk": np.ascontiguousarray(A_qk.astype(BF)),
        "WvT": np.ascontiguousarray(np.asarray(Wv, np.float32).T.astype(BF)),
        "W1T": np.ascontiguousarray((W1f * gf[None, :]).T.astype(BF)),
        "W2T": np.ascontiguousarray(np.asarray(W2, np.float32).T.astype(BF)),
        "b2": np.asarray(b2, np.float32),
        "r1": W1f @ gf,
        "w1bb1": W1f @ np.asarray(ln_b, np.float32) + np.asarray(b1, np.float32),
        "ones128": np.ones(128, np.float32),
    }
    in_maps = []
    for c in range(N_CORES):
        b, h = divmod(c, 2)
        xT = x[:, b, :].T  # (512, 2048)
        q = xT[:, h * SQ:(h + 1) * SQ]
        o = xT[:, (1 - h) * SQ:(2 - h) * SQ]
        xp = np.concatenate([q, o], axis=1)  # (512, 2048), q-half first
        in_maps.append({"xT": np.ascontiguousarray(xp.astype(BF)),
                        "xTM": np.ascontiguousarray(xp.T.astype(BF)),
                        **shared})
    return in_maps


def kernel(x, Wq, Wk, Wv, ln_g, ln_b, W1, b1, W2, b2):
    nc = _get_nc()
    in_maps = make_in_maps(x, Wq, Wk, Wv, ln_g, ln_b, W1, b1, W2, b2)
    res = run_bass_kernel_spmd(nc, in_maps, list(range(N_CORES)))
    out = np.empty((S, B, D), dtype=np.float32)
    for c in range(N_CORES):
        b, h = divmod(c, 2)
        out[h * SQ:(h + 1) * SQ, b, :] = res.results[c]["outT"].T.astype(np.float32)
    return out



# revision 61
# speedup vs baseline: 1.0214x; 1.0160x over previous
"""Trainium2 Bass kernel for the attention+LN+MLP block (nn_Attention_84310208020626).

Reference computation (per batch b):
    q = x_b @ Wq.T ; k = x_b @ Wk.T ; v = x_b @ Wv.T          (S=2048, D=512)
    attn = softmax(q k^T / sqrt(512))
    res  = attn @ v
    h    = LayerNorm(res) * ln_g + ln_b
    out  = relu(h @ W1.T + b1) @ W2.T + b2

Sharding: 8 cores = 4 batches x 2 sequence halves. Every core computes its
batch's full K/V (recompute, no collectives) and runs attention + LN + MLP
for its own 1024 query rows.

Device layout: activations are feature-major [feature, seq] so that every
GEMM contracts over the partition dimension without transposes:
    GT[d',s]     = A-stationary GEMM over xT, A = Wq^T Wk precomputed on host
                   (scores = q k^T = (x A) x^T, so no separate Q/K GEMMs)
    scoresT[t,s] = xT-stationary GEMM, rhs = GT     -> exp -> expT (bf16)
    Z[d,s]       = xTM-stationary GEMM over expT; resU[e,s] = WvT @ Z
LayerNorm sums over e and the softmax denominator use DVE pairwise-add
trees followed by a single ones-stationary matmul each; the softmax
division is folded into LN via scale invariance with a corrected epsilon:
    LN(res) = (resU - muU) / sqrt(varU + eps*sums^2)  (exact in exact arithmetic)
and the whole LN is folded into the MLP1 GEMM epilogue:
    h1 = relu( (G1 @ res - muU[s]*r1[f])*rstd[s] + (W1@ln_b)[f] + b1[f] )
where the rank-1 term -muU[s]*r1[f] is accumulated into the P PSUM by a
K=1 matmul (stationary = r1 row) and rstd is broadcast across partitions
with a K=1 ones matmul. G1 = W1*diag(ln_g), r1 = W1 @ ln_g and
w1bb1 = W1 @ ln_b + b1 are precomputed on the host (like A_qk).
All GEMM operands are bf16 (fp32 PSUM accumulation); LN stats math is fp32
with a fast-approximate reciprocal (~18 bits) for 1/sqrt(var).
Input DMAs are staged in first-use order across the three DMA queues
(sync/scalar/gpsimd); A is repacked ec-major on the host so the GT GEMM's
first stationary group is the first 128KB off the wire.
"""

import ml_dtypes
import numpy as np

import concourse.bass as bass
import concourse.mybir as mybir
import concourse.tile as tile
from concourse import bacc
from concourse.bass_utils import run_bass_kernel_spmd

S, B, D = 2048, 4, 512
N_CORES = 8
SQ = 1024          # query rows per core
SBLK = 512         # s-block (pipeline granularity)
NBLK = SQ // SBLK  # 2
ND = D // 128      # 4 chunks of the feature dims
NT = S // 128      # 16 t-chunks
NTT = S // 512     # 4 t-tiles of 512 for KT GEMM
EPS = 1e-5
SCALE = 1.0 / float(np.sqrt(512.0))

F32 = mybir.dt.float32
F32R = mybir.dt.float32r
BF16 = mybir.dt.bfloat16
AF = mybir.ActivationFunctionType
ALU = mybir.AluOpType


def _emit(nc, tc, n_iters=1):
    xT = nc.tensor_by_name["xT"].ap()       # (512, 2048) bf16, q-half first
    xTM = nc.tensor_by_name["xTM"].ap()     # (2048, 512) bf16, same t order
    A_qk = nc.tensor_by_name["A_qk"].ap()   # (512, 512) = Wq.T @ Wk  (d, d')
    WvT = nc.tensor_by_name["WvT"].ap()
    W1T = nc.tensor_by_name["W1T"].ap()     # (512, 512) = (W1*ln_g).T  (e, f)
    W2T = nc.tensor_by_name["W2T"].ap()
    b2 = nc.tensor_by_name["b2"].ap()
    r1 = nc.tensor_by_name["r1"].ap()       # (512,) = W1 @ ln_g
    w1bb1 = nc.tensor_by_name["w1bb1"].ap()  # (512,) = W1 @ ln_b + b1
    outT = nc.tensor_by_name["outT"].ap()   # (512, 1024) fp32 out

    # ---------------- SBUF tiles ----------------
    from contextlib import ExitStack
    ctx = ExitStack()
    consts = ctx.enter_context(tc.tile_pool(name="consts", bufs=1))
    big = ctx.enter_context(tc.tile_pool(name="big", bufs=1))
    qt_pool = ctx.enter_context(tc.tile_pool(name="qt", bufs=2))
    exp_pool = ctx.enter_context(tc.tile_pool(name="expp", bufs=2))
    res_pool = ctx.enter_context(tc.tile_pool(name="resp", bufs=2))
    h1_pool = ctx.enter_context(tc.tile_pool(name="h1p", bufs=2))
    out_pool = ctx.enter_context(tc.tile_pool(name="outp", bufs=2))
    sq_pool = ctx.enter_context(tc.tile_pool(name="sqp", bufs=4))
    row_pool = ctx.enter_context(tc.tile_pool(name="rowp", bufs=2))
    bc_pool = ctx.enter_context(tc.tile_pool(name="bcp", bufs=2))

    mm_psum = ctx.enter_context(tc.tile_pool(name="mmps", bufs=8, space="PSUM"))

    # constants / weights (W1T is pre-scaled by ln_g on the host; r1 and
    # w1bb1 = W1@ln_b + b1 are precomputed host-side as well)
    # A arrives ec-major (host-repacked): stationary chunk = a_sb[:, ec, dc, :]
    a_sb = consts.tile([128, ND, ND, 128], BF16)  # (p, ec, dc, e)
    wv_sb = consts.tile([128, ND, D], BF16)
    w1_sb = consts.tile([128, ND, D], BF16)
    w2_sb = consts.tile([128, ND, D], BF16)
    b2_sb = consts.tile([128, ND], F32)
    r1row = consts.tile([1, D], F32R)   # r1 on one partition (rank-1 stationary)
    w1bb1_sb = consts.tile([128, ND], F32)
    # Input DMAs in need-order, spread over the three DMA queues
    # (sync / scalar / gpsimd) so transfers overlap and the GT GEMM can
    # start as early as possible.
    # Wave 1: the GT GEMM consumes (a_dc, x_dc) pairs in dc order at ~1us
    # cadence, and both GT s-blocks read the full query half [0:1024).
    # 2KB-per-partition runs halve the DMA packet count vs 1KB.
    x_sb = big.tile([128, ND, S], BF16, tag="x", name="x_sb")
    ar = A_qk.rearrange("(ec p) (dc e) -> p ec dc e", p=128, e=128)
    xr = xT.rearrange("(dc p) t -> p dc t", p=128)
    xtm_sb = big.tile([128, NT, D], BF16, tag="v", name="xtm_sb")
    xmr = xTM.rearrange("(tc p) d -> p tc d", p=128)
    wvr = WvT.rearrange("(dc p) e -> p dc e", p=128)
    wr1 = W1T.rearrange("(dc p) e -> p dc e", p=128)
    wr2 = W2T.rearrange("(dc p) e -> p dc e", p=128)

    def xq(dc, h):  # quarter-tile of x: 128KB, 1KB runs
        return (x_sb[:, dc, h * 512:(h + 1) * 512],
                xr[:, dc, h * 512:(h + 1) * 512])

    def xo(dc):  # other-half tile of x: 256KB, 2KB runs
        return (x_sb[:, dc, 1024:2048], xr[:, dc, 1024:2048])

    def xtm(g):
        return (xtm_sb[:, 4 * g:4 * (g + 1), :], xmr[:, 4 * g:4 * (g + 1), :])

    # Per-queue issue order matches first-use order on the PE:
    # GT sb0 consumes a ec-group 0 + x[0:512] chunks first, GT sb1
    # x[512:1024], the scores GEMM then reads x[:,1024:2048], then xtm etc.
    def aq(ec):
        return (a_sb[:, ec, :, :], ar[:, ec, :, :])

    sync_q = [xq(0, 0), xq(2, 0), xq(0, 1), xq(2, 1),
              xo(0), xtm(0), xtm(3)]
    scalar_q = [aq(0), aq(1), aq(2), aq(3), xo(1), xtm(1),
                (wv_sb[:, :, :], wvr[:, :, :]),
                (w2_sb[:, :, :], wr2[:, :, :])]
    gpsimd_q = [xq(1, 0), xq(3, 0), xq(1, 1), xq(3, 1),
                xo(2), xo(3), xtm(2)]
    gpsimd_q += [(v_sb[:, :], v_dram.rearrange("(c p) -> p c", p=128))
                 for v_sb, v_dram in ((b2_sb, b2), (w1bb1_sb, w1bb1))]
    gpsimd_q.append((r1row[:, :],
                     r1.bitcast(F32R).rearrange("(c e) -> c e", c=1)))
    gpsimd_q.append((w1_sb[:, :, :], wr1[:, :, :]))
    for eng, q in ((nc.sync, sync_q), (nc.scalar, scalar_q),
                   (nc.gpsimd, gpsimd_q)):
        for dst, src in q:
            eng.dma_start(out=dst, in_=src)

    ones128 = nc.tensor_by_name["ones128"].ap()  # (128,) of 1.0
    ones_col_b = consts.tile([128, 1], BF16)   # stationary for column sums
    nc.vector.memset(ones_col_b, 1.0)
    ones_row = consts.tile([1, 128], F32R)      # stationary for partition broadcast
    nc.gpsimd.dma_start(out=ones_row[:, :],
                        in_=ones128.bitcast(F32R).rearrange("(c p) -> c p", c=1))

    for _iter in range(n_iters):
        _emit_iter(nc, tc, x_sb, xtm_sb, outT, big, qt_pool, exp_pool, res_pool,
                   h1_pool, out_pool, sq_pool, row_pool, bc_pool, mm_psum,
                   a_sb, wv_sb, w1_sb, w2_sb, b2_sb,
                   ones_col_b, ones_row, r1row, w1bb1_sb)

    ctx.close()


def _emit_iter(nc, tc, x_sb, xtm_sb, outT, big, qt_pool, exp_pool, res_pool,
               h1_pool, out_pool, sq_pool, row_pool, bc_pool, mm_psum,
               a_sb, wv_sb, w1_sb, w2_sb, b2_sb,
               ones_col_b, ones_row, r1row, w1bb1_sb):
    # ------- GT = A-stationary GEMM (G = x @ A; scores = G @ x^T) -------
    qt_tiles = []
    for sb in range(NBLK):
        s0 = sb * SBLK
        qt_sb = qt_pool.tile([128, ND, SBLK], BF16, tag="qt")
        for ec in range(ND):
            qps = mm_psum.tile([128, 512], F32, tag="mm")
            for dc in range(ND):
                nc.tensor.matmul(
                    qps[:, :],
                    a_sb[:, ec, dc, :],
                    x_sb[:, dc, s0:s0 + SBLK],
                    start=(dc == 0), stop=(dc == ND - 1),
                )
            nc.scalar.copy(out=qt_sb[:, ec, :], in_=qps[:, :])
        qt_tiles.append(qt_sb)

    # ---------------- per s-block pipeline (software-pipelined emission) ----
    # emission order: scores(0), res(0), scores(1), stats(0), res(1),
    # norm+mlp(0), stats(1), norm+mlp(1) - keeps matmul work queued on PE
    # while DVE/ACT compute the LN row stats of the previous block.
    exp_tiles = [None] * NBLK
    esum_tiles = [None] * NBLK
    res_tiles = [None] * NBLK
    rows2_tiles = [None] * NBLK

    def emit_scores(sb):
        qt_sb = qt_tiles[sb]
        exp_sb = exp_pool.tile([128, NT, SBLK], BF16, tag="exp", name=f"exp{sb}")
        for tc_i in range(NT):
            sps = mm_psum.tile([128, 512], F32, tag="mm")
            for dc in range(ND):
                nc.tensor.matmul(
                    sps[:, :],
                    x_sb[:, dc, tc_i * 128:(tc_i + 1) * 128],
                    qt_sb[:, dc, :],
                    start=(dc == 0), stop=(dc == ND - 1),
                )
            nc.scalar.activation(out=exp_sb[:, tc_i, :], in_=sps[:, :],
                                 func=AF.Exp, scale=SCALE)
        exp_tiles[sb] = exp_sb
        # DVE pairwise-add tree over the 16 t-chunks: trails the exp ACTs
        # while PE streams the scores GEMM, so the softmax-denominator
        # reduction needs a single ones-matmul instead of 16.
        es = exp_pool.tile([128, 8, SBLK], BF16, tag="es", name=f"es{sb}")
        for j in range(8):
            nc.vector.tensor_add(out=es[:, j, :], in0=exp_sb[:, 2 * j, :],
                                 in1=exp_sb[:, 2 * j + 1, :])
        for lvl in (4, 2, 1):
            for j in range(lvl):
                nc.vector.tensor_add(out=es[:, j, :], in0=es[:, 2 * j, :],
                                     in1=es[:, 2 * j + 1, :])
        esum_tiles[sb] = es

    def emit_res(sb):
        exp_sb = exp_tiles[sb]
        # Z[d, s] = sum_t x[t,d] * exp[t,s]   (x t-major stationary)
        z_sb = sq_pool.tile([128, ND, SBLK], BF16, tag="z", name=f"z{sb}")
        for dc in range(ND):
            zps = mm_psum.tile([128, 512], F32, tag="mm")
            for tc_i in range(NT):
                nc.tensor.matmul(
                    zps[:, :],
                    xtm_sb[:, tc_i, dc * 128:(dc + 1) * 128],
                    exp_sb[:, tc_i, :],
                    start=(tc_i == 0), stop=(tc_i == NT - 1),
                )
            nc.scalar.copy(out=z_sb[:, dc, :], in_=zps[:, :])
        # resU[e, s] = Wv @ Z
        res_sb = res_pool.tile([128, ND, SBLK], BF16, tag="res", name=f"res{sb}")
        for ec in range(ND):
            rps = mm_psum.tile([128, 512], F32, tag="mm")
            for dc in range(ND):
                nc.tensor.matmul(
                    rps[:, :],
                    wv_sb[:, dc, ec * 128:(ec + 1) * 128],
                    z_sb[:, dc, :],
                    start=(dc == 0), stop=(dc == ND - 1),
                )
            nc.scalar.copy(out=res_sb[:, ec, :], in_=rps[:, :])
        res_tiles[sb] = res_sb

    rt_tiles = [None] * NBLK
    sq_tiles = [None] * NBLK

    def emit_stats_dve(sb):
        # DVE add-trees over the 4 e-chunks for sum(res) and sum(res^2)
        res_sb = res_tiles[sb]
        rt = sq_pool.tile([128, 2, SBLK], BF16, tag="rt", name=f"rt{sb}")
        nc.vector.tensor_add(out=rt[:, 0, :], in0=res_sb[:, 0, :],
                             in1=res_sb[:, 1, :])
        nc.vector.tensor_add(out=rt[:, 1, :], in0=res_sb[:, 2, :],
                             in1=res_sb[:, 3, :])
        nc.vector.tensor_add(out=rt[:, 0, :], in0=rt[:, 0, :], in1=rt[:, 1, :])
        # squares on ACT (has slack here) to unload the DVE; adds stay DVE
        sq_sb = sq_pool.tile([128, ND, SBLK], BF16, tag="sq", name=f"sq{sb}")
        for ec in range(ND):
            nc.scalar.square(out=sq_sb[:, ec, :], in_=res_sb[:, ec, :])
        nc.vector.tensor_add(out=sq_sb[:, 0, :], in0=sq_sb[:, 0, :],
                             in1=sq_sb[:, 1, :])
        nc.vector.tensor_add(out=sq_sb[:, 2, :], in0=sq_sb[:, 2, :],
                             in1=sq_sb[:, 3, :])
        nc.vector.tensor_add(out=sq_sb[:, 0, :], in0=sq_sb[:, 0, :],
                             in1=sq_sb[:, 2, :])
        rt_tiles[sb] = rt
        sq_tiles[sb] = sq_sb

    def emit_stats(sb):
        sums_ps = mm_psum.tile([1, 512], F32, tag="mm")
        nc.tensor.matmul(sums_ps[:, :], ones_col_b[:, :],
                         esum_tiles[sb][:, 0, :], start=True, stop=True)
        sume_ps = mm_psum.tile([1, 512], F32, tag="mm")
        nc.tensor.matmul(sume_ps[:, :], ones_col_b[:, :],
                         rt_tiles[sb][:, 0, :], start=True, stop=True)
        sumsq_ps = mm_psum.tile([1, 512], F32, tag="mm")
        nc.tensor.matmul(sumsq_ps[:, :], ones_col_b[:, :],
                         sq_tiles[sb][:, 0, :], start=True, stop=True)

        # row stats on one partition (all on DVE where [1,512] ops are ~140ns
        # vs ~680ns on ACT; only sqrt stays on ACT):
        #   muU = sumE/512 ; varU = sumSq/512 - muU^2
        #   rstd = 1/sqrt(varU + EPS*sums^2) ; murstd = muU*rstd
        rows = row_pool.tile([1, 4, SBLK], F32, tag="rows", name=f"rows{sb}")
        rows2 = row_pool.tile([1, 2, SBLK], F32R, tag="rows2", name=f"rows2{sb}")
        nc.vector.tensor_scalar_mul(out=rows[:, 0, :], in0=sume_ps[:, :],
                                    scalar1=-1.0 / D)                        # -muU
        # -muU feeds the P GEMM's rank-1 accumulation: publish it first so
        # the PE doesn't wait on the sqrt/reciprocal part of this chain
        nc.vector.tensor_copy(out=rows2[:, 1, :], in_=rows[:, 0, :])         # -muU
        nc.scalar.activation(out=rows[:, 2, :], in_=sums_ps[:, :],
                             func=AF.Square, scale=float(np.sqrt(EPS)))      # eps*sums^2
        nc.vector.tensor_mul(out=rows[:, 3, :], in0=rows[:, 0, :], in1=rows[:, 0, :])
        nc.vector.scalar_tensor_tensor(
            out=rows[:, 1, :], in0=sumsq_ps[:, :], scalar=1.0 / D,
            in1=rows[:, 3, :], op0=ALU.mult, op1=ALU.subtract)               # msq-muU^2
        nc.vector.tensor_add(out=rows[:, 1, :], in0=rows[:, 1, :], in1=rows[:, 2, :])
        nc.scalar.activation(out=rows[:, 1, :], in_=rows[:, 1, :], func=AF.Sqrt)
        nc.vector.reciprocal_approx_fast(out=rows[:, 3, :], in_=rows[:, 1, :])
        nc.vector.tensor_copy(out=rows2[:, 0, :], in_=rows[:, 3, :])         # rstd
        rows2_tiles[sb] = rows2

    p_tiles = [None] * NBLK
    h1_tiles = [None] * NBLK

    def emit_p(sb):
        # P = G1 @ res, plus a rank-1 matmul accumulating murstd[s]*r1[f]
        # into the same PSUM (replaces a per-fc DVE scalar_tensor_tensor in
        # the epilogue; the DVE queue is the end-phase bottleneck).
        res_sb = res_tiles[sb]
        rows2 = rows2_tiles[sb]
        p_ps = []
        for fc in range(ND):
            hps = mm_psum.tile([128, 512], F32, tag="mm", name=f"p{sb}_{fc}")
            for ec in range(ND):
                nc.tensor.matmul(
                    hps[:, :],
                    w1_sb[:, ec, fc * 128:(fc + 1) * 128],
                    res_sb[:, ec, :],
                    start=(ec == 0), stop=False,
                )
            nc.tensor.matmul(
                hps[:, :], r1row[:, fc * 128:(fc + 1) * 128],
                rows2[:, 1, :], start=False, stop=True,
            )
            p_ps.append(hps)
        p_tiles[sb] = p_ps

    def emit_bc_epi(sb):
        rows2 = rows2_tiles[sb]
        p_ps = p_tiles[sb]

        # broadcast rstd across 128 partitions via a K=1 matmul
        bc_sb = bc_pool.tile([128, SBLK], F32, tag="bc_sb")
        bc_ps = mm_psum.tile([128, 512], F32, tag="mm")
        nc.tensor.matmul(
            bc_ps[:, :], ones_row[:, :],
            rows2[:, 0, :], start=True, stop=True,
        )
        nc.scalar.copy(out=bc_sb[:, :], in_=bc_ps[:, :])

        # fused MLP1 + LayerNorm epilogue (murstd*r1 already accumulated
        # into the P psum by emit_p):
        #   h1 = relu( (P - murstd[s]*r1[f])*rstd[s] ... wait, P already
        #   holds G1@res + murstd*r1; h1 = relu(P*rstd + w1b[f] + b1[f])
        h1_sb = h1_pool.tile([128, ND, SBLK], BF16, tag="h1", name=f"h1_{sb}")
        for fc in range(ND):
            t_sb = sq_pool.tile([128, SBLK], F32R, tag="sq")
            nc.vector.tensor_mul(out=t_sb[:, :], in0=p_ps[fc][:, :],
                                 in1=bc_sb[:, :])
            nc.scalar.activation(out=h1_sb[:, fc, :], in_=t_sb[:, :],
                                 func=AF.Relu, bias=w1bb1_sb[:, fc:fc + 1])
        h1_tiles[sb] = h1_sb

    def emit_mlp2(sb):
        s0 = sb * SBLK
        h1_sb = h1_tiles[sb]
        o_sb = out_pool.tile([128, ND, SBLK], BF16, tag="o")
        outr = outT[:, s0:s0 + SBLK].rearrange("(gc p) s -> p gc s", p=128)
        for gc in range(ND):
            ops = mm_psum.tile([128, 512], F32, tag="mm")
            for fc in range(ND):
                nc.tensor.matmul(
                    ops[:, :],
                    w2_sb[:, fc, gc * 128:(gc + 1) * 128],
                    h1_sb[:, fc, :],
                    start=(fc == 0), stop=(fc == ND - 1),
                )
            # alternate the bias epilogue between ACT and DVE so the final
            # chunks drain in parallel instead of serializing on one engine
            if gc % 2 == 0:
                nc.scalar.activation(out=o_sb[:, gc, :], in_=ops[:, :],
                                     func=AF.Identity, bias=b2_sb[:, gc:gc + 1])
            else:
                nc.vector.tensor_scalar_add(out=o_sb[:, gc, :], in0=ops[:, :],
                                            scalar1=b2_sb[:, gc:gc + 1])
            eng = (nc.sync, nc.scalar, nc.gpsimd, nc.sync)[gc]
            eng.dma_start(out=outr[:, gc, :], in_=o_sb[:, gc, :])

    emit_scores(0)
    emit_res(0)
    emit_stats_dve(0)
    emit_scores(1)
    emit_stats(0)
    emit_res(1)
    emit_stats_dve(1)
    emit_p(0)
    emit_stats(1)
    emit_bc_epi(0)
    emit_p(1)
    emit_bc_epi(1)
    emit_mlp2(0)
    emit_mlp2(1)


def build_nc(n_iters=1):
    nc = bacc.Bacc("TRN2", target_bir_lowering=False, debug=False)
    nc.tensor_by_name = {}

    def dram(name, shape, kind):
        t = nc.dram_tensor(name, shape, F32, kind=kind)
        nc.tensor_by_name[name] = t
        return t

    def dram_bf(name, shape, kind):
        t = nc.dram_tensor(name, shape, BF16, kind=kind)
        nc.tensor_by_name[name] = t
        return t

    dram_bf("xT", [D, S], "ExternalInput")
    dram_bf("xTM", [S, D], "ExternalInput")
    for nm in ("A_qk", "WvT", "W1T", "W2T"):
        dram_bf(nm, [D, D], "ExternalInput")
    for nm in ("b2", "r1", "w1bb1"):
        dram(nm, [D], "ExternalInput")
    dram("ones128", [128], "ExternalInput")
    dram_bf("outT", [D, SQ], "ExternalOutput")

    with tile.TileContext(nc) as tc:
        _emit(nc, tc, n_iters=n_iters)
    nc.compile()
    return nc


_CACHED_NC = None


def _get_nc():
    global _CACHED_NC
    if _CACHED_NC is None:
        _CACHED_NC = build_nc()
    return _CACHED_NC


def make_in_maps(x, Wq, Wk, Wv, ln_g, ln_b, W1, b1, W2, b2):
    BF = ml_dtypes.bfloat16
    x = np.asarray(x, dtype=np.float32)
    A_qk = np.asarray(Wq, np.float32).T @ np.asarray(Wk, np.float32)
    # repack ec-major: A2[ec*128+p, dc*128+e] = A_qk[dc*128+p, ec*128+e]
    A_qk = np.ascontiguousarray(
        A_qk.reshape(4, 128, 4, 128).transpose(2, 1, 0, 3).reshape(512, 512))
    W1f = np.asarray(W1, np.float32)
    gf = np.asarray(ln_g, np.float32)
    # LayerNorm fold, precomputed host-side:
    #   G1 = W1 * ln_g[e]  (pre-scaled MLP1 weight)
    #   r1 = W1 @ ln_g     (rank-1 -mu*rstd correction row)
    #   w1bb1 = W1 @ ln_b + b1
    shared = {
        "A_qk": np.ascontiguousarray(A_qk.astype(BF)),
        "WvT": np.ascontiguousarray(np.asarray(Wv, np.float32).T.astype(BF)),
        "W1T": np.ascontiguousarray((W1f * gf[None, :]).T.astype(BF)),
        "W2T": np.ascontiguousarray(np.asarray(W2, np.float32).T.astype(BF)),
        "b2": np.asarray(b2, np.float32),
        "r1": W1f @ gf,
        "w1bb1": W1f @ np.asarray(ln_b, np.float32) + np.asarray(b1, np.float32),
        "ones128": np.ones(128, np.float32),
    }
    in_maps = []
    for c in range(N_CORES):
        b, h = divmod(c, 2)
        xT = x[:, b, :].T  # (512, 2048)
        q = xT[:, h * SQ:(h + 1) * SQ]
        o = xT[:, (1 - h) * SQ:(2 - h) * SQ]
        xp = np.concatenate([q, o], axis=1)  # (512, 2048), q-half first
        in_maps.append({"xT": np.ascontiguousarray(xp.astype(BF)),
                        "xTM": np.ascontiguousarray(xp.T.astype(BF)),
                        **shared})
    return in_maps


def kernel(x, Wq, Wk, Wv, ln_g, ln_b, W1, b1, W2, b2):
    nc = _get_nc()
    in_maps = make_in_maps(x, Wq, Wk, Wv, ln_g, ln_b, W1, b1, W2, b2)
    res = run_bass_kernel_spmd(nc, in_maps, list(range(N_CORES)))
    out = np.empty((S, B, D), dtype=np.float32)
    for c in range(N_CORES):
        b, h = divmod(c, 2)
        out[h * SQ:(h + 1) * SQ, b, :] = res.results[c]["outT"].T.astype(np.float32)
    return out

